# revision 33
# baseline (speedup 1.0000x reference)
"""Trainium2 Bass kernel for nn_NeuralMemory (scatter_memory).

Math: the reference's einsum 'bdd->bd' reads only the DIAGONAL of M, so the
sequential part reduces to a 2-state (diag M, diag S) linear recurrence per
(b, d) lane.  Outputs y_t = q_t * diagM^(t); the full M/S finals are weighted
sums of outer products  sum_t w_t * k_t (err_t)^T  == matmuls.

Per core (1 batch), S=1024 timesteps are split into C=128 chunks of L=8.
 - phase A: per-chunk affine coefficient systems, 3 systems concatenated as
   PR=[phi|psi|rho], MS=[mu|nu|sigma] [C, 3D]; chunks on partitions so the
   per-step gate scalars fuse via scalar_tensor_tensor.
 - phase B: chunk maps composed across chunks: level-2 (8 -> 16 superchunk
   maps, free-concatenated 2x2 map algebra), level-3 (15 sequential superchunk
   steps), then reconstruction of all 128 chunk-entry states.
 - phase C: d^(t) for every t from saved coefficients; y/err elementwise.
 - finals: S_F/M_F as PSUM-accumulated matmuls with per-t scalar weights from
   tensor_tensor_scan backward recurrences (cS, Ba, m), truncated to the last
   TW steps (decay products underflow f32 long before that; verified 1e-169).

Sharding: pure data parallel, batch b -> core b.  Host passes x pre-transposed
([d,t] per core), a single concatenated constants array, replicated weights.
"""
import numpy as np
from contextlib import ExitStack

import concourse.bass as bass
import concourse.tile as tile
from concourse import bacc, mybir
from concourse.bass_utils import run_bass_kernel_spmd

F32 = mybir.dt.float32
AF = mybir.ActivationFunctionType
OP = mybir.AluOpType

B, S, D = 8, 1024, 128
L, C = 8, 128          # chunk length, chunks (C on partitions)
L2, C2 = 8, 16         # level-2: superchunks of 8 chunks
TW = 512               # final-state weight window (last TW timesteps)
CW = C - TW // L       # first chunk row with nonzero weights


def _mk_tile(pool, shape, dtype, tag):
    return pool.tile(shape, dtype, tag=tag, name=tag)


def emit(ctx: ExitStack, tc: tile.TileContext, outs, ins, sfx=""):
    nc = tc.nc
    y_d, mf_d, sf_d = outs
    xT_d, consts_d = ins

    p = ctx.enter_context(tc.tile_pool(name="main" + sfx, bufs=1))
    pt = ctx.enter_context(tc.tile_pool(name="tmp" + sfx, bufs=3))
    ps = ctx.enter_context(tc.tile_pool(name="psum" + sfx, bufs=2, space="PSUM"))
    psacc = ctx.enter_context(tc.tile_pool(name="psacc" + sfx, bufs=1, space="PSUM"))

    # ---------------- input DMAs ----------------
    NCONST = 4 * D + 3 + 3 * L
    cst = _mk_tile(p, [D, NCONST], F32, "cst")
    nc.sync.dma_start(cst[:], consts_d[:])
    xT = _mk_tile(p, [D, S], F32, "xT")
    for jj in range(4):
        nc.sync.dma_start(xT[:, jj * 256:(jj + 1) * 256],
                          xT_d[:, jj * 256:(jj + 1) * 256])
    wk = cst[:, 0:D]
    wv = cst[:, D:2 * D]
    wq = cst[:, 2 * D:3 * D]
    ident = cst[:, 3 * D:4 * D]
    wg = cst[:, 4 * D:4 * D + 3]
    bgb = cst[:, 4 * D + 3:NCONST]

    # ---------------- gates ----------------
    # prime the ACT exp/ln table and warm up the PE while inputs stream in
    prime = _mk_tile(pt, [1, 1], F32, "prime")
    nc.scalar.activation(prime[:], cst[0:1, 0:1], AF.Exp, scale=1.0)
    pwm = _mk_tile(ps, [D, D], F32, "mm")
    nc.tensor.matmul(pwm[:], cst[:, 3 * D:4 * D], cst[:, 3 * D:4 * D],
                     start=True, stop=True)
    gp = _mk_tile(ps, [C, 3 * L], F32, "mm")
    for i in range(L):
        nc.tensor.matmul(gp[:, i * 3:(i + 1) * 3], xT[:, i::L], wg,
                         start=True, stop=True)
    zg = _mk_tile(p, [C, 3 * L], F32, "zg")
    nc.vector.tensor_tensor(zg[:], gp[:], bgb, OP.add)
    # gates = [a' | e | th] at cols {3i+0,3i+1,3i+2}
    eg = _mk_tile(p, [C, 3 * L], F32, "eg")
    zg3 = zg[:].rearrange("p (i g) -> p i g", g=3)
    eg3 = eg[:].rearrange("p (i g) -> p i g", g=3)
    nc.scalar.activation(eg3[:, :, 0:1], zg3[:, :, 0:1], AF.Exp, scale=1.0)
    nc.scalar.activation(eg3[:, :, 1:3], zg3[:, :, 1:3], AF.Exp, scale=-1.0)
    den = _mk_tile(pt, [C, 3 * L], F32, "den")
    nc.vector.tensor_scalar_add(den[:], eg[:], 1.0)
    gates = _mk_tile(p, [C, 3 * L], F32, "gates")
    nc.vector.reciprocal(gates[:], den[:])
    ga = gates[:].rearrange("p (i g) -> p i g", g=3)

    def a_col(i):
        return ga[:, i, 0:1]

    def e_col(i):
        return ga[:, i, 1:2]

    def th_col(i):
        return ga[:, i, 2:3]

    # ------- projections: kvq slabs [k|v|q] per i, then cw slabs [c|w] -------
    kvq = _mk_tile(p, [C, L * 3 * D], F32, "kvq")

    def k_sl(i):
        return kvq[:, i * 3 * D:i * 3 * D + D]

    def v_sl(i):
        return kvq[:, i * 3 * D + D:i * 3 * D + 2 * D]

    def q_sl(i):
        return kvq[:, i * 3 * D + 2 * D:i * 3 * D + 3 * D]

    for i in range(L):
        mm = _mk_tile(ps, [C, 3 * D], F32, "mm")
        nc.tensor.matmul(mm[:, 0:D], xT[:, i::L], wk, start=True, stop=True)
        nc.tensor.matmul(mm[:, D:2 * D], xT[:, i::L], wv, start=True, stop=True)
        nc.tensor.matmul(mm[:, 2 * D:3 * D], xT[:, i::L], wq, start=True, stop=True)
        nc.scalar.copy(kvq[:, i * 3 * D:(i + 1) * 3 * D], mm[:])

    cw = _mk_tile(p, [C, L * 2 * D], F32, "cw")

    def c_sl(i):
        return cw[:, i * 2 * D:i * 2 * D + D]

    def w_sl(i):
        return cw[:, i * 2 * D + D:i * 2 * D + 2 * D]

    for i in range(L):
        kth = _mk_tile(pt, [C, D], F32, "kth")
        nc.gpsimd.tensor_tensor(kth[:], th_col(i).broadcast_to((C, D)), k_sl(i),
                                OP.mult)
        kb2 = kth[:].unsqueeze(1).broadcast_to((C, 2, D))
        nc.gpsimd.tensor_tensor(
            cw[:, i * 2 * D:(i + 1) * 2 * D].rearrange("p (a b) -> p a b", a=2),
            kb2, kvq[:, i * 3 * D:i * 3 * D + 2 * D].rearrange("p (a b) -> p a b", a=2),
            OP.mult)

    # -------- phase A: PR2=[phi|psi], MS2=[mu|nu] on DVE chain; rho/sg small ----
    PR2 = [None] * (L + 1)
    MS2 = [None] * (L + 1)
    rho = [None] * (L + 1)
    sg = [None] * (L + 1)
    PR2[1] = _mk_tile(p, [C, 2 * D], F32, "PR1")
    MS2[1] = _mk_tile(p, [C, 2 * D], F32, "MS1")
    rho[1] = _mk_tile(p, [C, D], F32, "rho1")
    sg[1] = _mk_tile(p, [C, D], F32, "sg1")
    nc.vector.tensor_scalar(PR2[1][:, 0:D], c_sl(0), -1.0, a_col(0), OP.mult, OP.add)
    nc.scalar.activation(PR2[1][:, D:2 * D], c_sl(0), AF.Identity, bias=e_col(0), scale=0.0)
    nc.vector.tensor_scalar_mul(MS2[1][:, 0:D], c_sl(0), -1.0)
    nc.scalar.activation(MS2[1][:, D:2 * D], c_sl(0), AF.Identity, bias=e_col(0), scale=0.0)
    nc.scalar.copy(rho[1][:], w_sl(0))
    nc.scalar.copy(sg[1][:], w_sl(0))
    for j in range(2, L + 1):
        i = j - 1
        PR2[j] = _mk_tile(p, [C, 2 * D], F32, f"PR{j}")
        MS2[j] = _mk_tile(p, [C, 2 * D], F32, f"MS{j}")
        rho[j] = _mk_tile(p, [C, D], F32, f"rho{j}")
        sg[j] = _mk_tile(p, [C, D], F32, f"sg{j}")
        # Pool: off-chain product T3 = c*rho - w
        t3p = _mk_tile(pt, [C, D], F32, "t3p")
        nc.gpsimd.tensor_tensor(t3p[:], c_sl(i), rho[j - 1][:], OP.mult)
        nc.gpsimd.tensor_tensor(t3p[:], t3p[:], w_sl(i), OP.subtract)
        # DVE serial chain
        cb2 = c_sl(i).unsqueeze(1).broadcast_to((C, 2, D))
        tA = _mk_tile(pt, [C, 2 * D], F32, "tA")
        nc.vector.tensor_tensor(tA[:].rearrange("p (a b) -> p a b", a=2), cb2,
                                PR2[j - 1][:].rearrange("p (a b) -> p a b", a=2),
                                OP.mult)
        nc.vector.scalar_tensor_tensor(MS2[j][:], MS2[j - 1][:], e_col(i), tA[:],
                                       OP.mult, OP.subtract)
        nc.vector.scalar_tensor_tensor(PR2[j][:], PR2[j - 1][:], a_col(i), MS2[j][:],
                                       OP.mult, OP.add)
        nc.vector.scalar_tensor_tensor(sg[j][:], sg[j - 1][:], e_col(i), t3p[:],
                                       OP.mult, OP.subtract)
        nc.vector.scalar_tensor_tensor(rho[j][:], rho[j - 1][:], a_col(i), sg[j][:],
                                       OP.mult, OP.add)

    # ---------------- transposes to [d, c] ----------------
    # PT_all blocks: 0=phiT 1=muT 2=psiT 3=nuT 4=rhoT 5=sgT
    ptall = _mk_tile(p, [D, 6 * C], F32, "ptall")
    srcs = [PR2[L][:, 0:D], MS2[L][:, 0:D], PR2[L][:, D:2 * D],
            MS2[L][:, D:2 * D], rho[L][:], sg[L][:]]
    for b_, src in enumerate(srcs):
        tp = _mk_tile(ps, [D, C], F32, "mm")
        nc.tensor.transpose(tp[:], src, ident)
        nc.scalar.copy(ptall[:, b_ * C:(b_ + 1) * C], tp[:])

    # ---------------- phase B level-2: superchunk map composition ----------------
    r6 = ptall[:].rearrange("p (b s j) -> p b s j", b=6, s=C2)
    Z = [None] * L2
    O_ = [None] * L2
    Z[0] = _mk_tile(p, [D, 4 * C2], F32, "Z0")
    O_[0] = _mk_tile(p, [D, 2 * C2], F32, "O0")
    nc.scalar.copy(Z[0][:, 0:2 * C2], r6[:, 0:3:2, :, 0])
    nc.scalar.copy(Z[0][:, 2 * C2:4 * C2], r6[:, 1:4:2, :, 0])
    nc.scalar.copy(O_[0][:], r6[:, 4:6, :, 0])
    for j in range(1, L2):
        Z[j] = _mk_tile(p, [D, 4 * C2], F32, f"Z{j}")
        O_[j] = _mk_tile(p, [D, 2 * C2], F32, f"O{j}")
        A4 = r6[:, 0:2, :, j].unsqueeze(2).broadcast_to((D, 2, 2, C2))
        B4 = r6[:, 2:4, :, j].unsqueeze(2).broadcast_to((D, 2, 2, C2))
        Zx = Z[j - 1][:, 0:2 * C2].unsqueeze(1).broadcast_to((D, 2, 2 * C2))
        Zy = Z[j - 1][:, 2 * C2:4 * C2].unsqueeze(1).broadcast_to((D, 2, 2 * C2))
        t1 = _mk_tile(pt, [D, 4 * C2], F32, "l2t1")
        t2 = _mk_tile(pt, [D, 4 * C2], F32, "l2t2")
        nc.vector.tensor_tensor(t1[:], A4, Zx, OP.mult)
        nc.vector.tensor_tensor(t2[:], B4, Zy, OP.mult)
        nc.vector.tensor_tensor(Z[j][:], t1[:], t2[:], OP.add)
        A2 = r6[:, 0:2, :, j]
        B2 = r6[:, 2:4, :, j]
        Q2 = r6[:, 4:6, :, j]
        o1b = O_[j - 1][:, 0:C2].unsqueeze(1).broadcast_to((D, 2, C2))
        o2b = O_[j - 1][:, C2:2 * C2].unsqueeze(1).broadcast_to((D, 2, C2))
        t3 = _mk_tile(pt, [D, 2 * C2], F32, "l2t3")
        t4 = _mk_tile(pt, [D, 2 * C2], F32, "l2t4")
        nc.gpsimd.tensor_tensor(t3[:], A2, o1b, OP.mult)
        nc.gpsimd.tensor_tensor(t4[:], B2, o2b, OP.mult)
        nc.gpsimd.tensor_tensor(t3[:], t3[:], t4[:], OP.add)
        nc.gpsimd.tensor_tensor(O_[j][:], t3[:], Q2, OP.add)

    # ------- scalar weights cS/m (last TW steps) via reversed TTS scans -------
    # (negative-stride DMAs wedge the device: forward gather DMA + negative-step
    #  compute-engine copies instead)
    tm3 = _mk_tile(p, [1, 3 * TW], F32, "tm3")   # [e | a | th] t-major, t>=S-TW
    nc.sync.dma_start(tm3[0:1, 0:TW], ga[CW:C, :, 1])
    nc.sync.dma_start(tm3[0:1, TW:2 * TW], ga[CW:C, :, 0])
    nc.sync.dma_start(tm3[0:1, 2 * TW:3 * TW], ga[CW:C, :, 2])
    erev = _mk_tile(p, [1, TW + 1], F32, "erev")
    arev = _mk_tile(p, [1, TW + 1], F32, "arev")
    threv = _mk_tile(p, [1, TW], F32, "threv")
    nc.gpsimd.memset(erev[0:1, 0:1], 1.0)
    nc.gpsimd.memset(arev[0:1, 0:1], 1.0)
    nc.vector.tensor_copy(erev[0:1, 1:TW + 1], tm3[0:1, 0:TW][:, ::-1])
    nc.scalar.copy(arev[0:1, 1:TW + 1], tm3[0:1, TW:2 * TW][:, ::-1])
    nc.scalar.copy(threv[:], tm3[0:1, 2 * TW:3 * TW][:, ::-1])
    zrow = _mk_tile(p, [1, TW], F32, "zrow")
    nc.gpsimd.memset(zrow[:], 0.0)
    barev = _mk_tile(p, [1, TW], F32, "barev")
    nc.vector.tensor_tensor_scan(barev[:], arev[0:1, 0:TW], zrow[:], 1.0, OP.mult, OP.add)
    csrev = _mk_tile(p, [1, TW], F32, "csrev")
    nc.vector.tensor_tensor_scan(csrev[:], erev[0:1, 0:TW], zrow[:], 1.0, OP.mult, OP.add)
    mrev = _mk_tile(p, [1, TW], F32, "mrev")
    nc.vector.tensor_tensor_scan(mrev[:], erev[0:1, 0:TW], barev[:], 0.0, OP.mult, OP.add)
    wsrev = _mk_tile(p, [1, TW], F32, "wsrev")
    nc.vector.scalar_tensor_tensor(wsrev[:], threv[:], -1.0, csrev[:], OP.mult, OP.mult)
    wmrev = _mk_tile(p, [1, TW], F32, "wmrev")
    nc.vector.scalar_tensor_tensor(wmrev[:], threv[:], -1.0, mrev[:], OP.mult, OP.mult)
    wstm = _mk_tile(p, [1, TW], F32, "wstm")
    wmtm = _mk_tile(p, [1, TW], F32, "wmtm")
    nc.scalar.copy(wstm[:], wsrev[0:1, ::-1])
    nc.scalar.copy(wmtm[:], wmrev[0:1, ::-1])
    wS = _mk_tile(p, [C, L], F32, "wS")
    wM = _mk_tile(p, [C, L], F32, "wM")
    nc.gpsimd.memset(wS[0:CW, :], 0.0)
    nc.gpsimd.memset(wM[0:CW, :], 0.0)
    nc.sync.dma_start(wS[CW:C, :], wstm[:])
    nc.sync.dma_start(wM[CW:C, :], wmtm[:])


    # ------- level-3: compose superchunk maps in sub-blocks of 4, then a
    # 3-step sequential walk over the 4 blocks, then in-block recon -------
    # Z[7] cols: q-blocks of C2; superchunk s = 4*s2 + j3
    L3, C3 = 4, 4
    zt = _mk_tile(p, [D, 2 * C], F32, "zt")
    nc.vector.memset(zt[:], 0.0)
    z7q = Z[L2 - 1][:].rearrange("p (q s2 j) -> p q s2 j", q=4, j=L3)
    o7q = O_[L2 - 1][:].rearrange("p (q s2 j) -> p q s2 j", q=2, j=L3)
    Z3 = [None] * L3
    O3 = [None] * L3
    Z3[0] = _mk_tile(p, [D, 4 * C3], F32, "Z30")
    O3[0] = _mk_tile(p, [D, 2 * C3], F32, "O30")
    nc.vector.tensor_copy(Z3[0][:].rearrange("p (q s2) -> p q s2", q=4),
                          z7q[:, :, :, 0])
    nc.gpsimd.tensor_copy(O3[0][:].rearrange("p (q s2) -> p q s2", q=2),
                          o7q[:, :, :, 0])
    for j3 in range(1, L3):
        Z3[j3] = _mk_tile(p, [D, 4 * C3], F32, f"Z3{j3}")
        O3[j3] = _mk_tile(p, [D, 2 * C3], F32, f"O3{j3}")
        A4 = z7q[:, 0:3:2, :, j3].unsqueeze(2).broadcast_to((D, 2, 2, C3))
        B4 = z7q[:, 1:4:2, :, j3].unsqueeze(2).broadcast_to((D, 2, 2, C3))
        Zx = Z3[j3 - 1][:, 0:2 * C3].unsqueeze(1).broadcast_to((D, 2, 2 * C3))
        Zy = Z3[j3 - 1][:, 2 * C3:4 * C3].unsqueeze(1).broadcast_to((D, 2, 2 * C3))
        t1 = _mk_tile(pt, [D, 4 * C3], F32, "l3ct1")
        t2 = _mk_tile(pt, [D, 4 * C3], F32, "l3ct2")
        nc.vector.tensor_tensor(t1[:], A4, Zx, OP.mult)
        nc.vector.tensor_tensor(t2[:], B4, Zy, OP.mult)
        nc.vector.tensor_tensor(Z3[j3][:], t1[:], t2[:], OP.add)
        A2 = z7q[:, 0:3:2, :, j3]
        B2 = z7q[:, 1:4:2, :, j3]
        o1b = O3[j3 - 1][:, 0:C3].unsqueeze(1).broadcast_to((D, 2, C3))
        o2b = O3[j3 - 1][:, C3:2 * C3].unsqueeze(1).broadcast_to((D, 2, C3))
        t3 = _mk_tile(pt, [D, 2 * C3], F32, "l3ct3")
        t4 = _mk_tile(pt, [D, 2 * C3], F32, "l3ct4")
        nc.gpsimd.tensor_tensor(t3[:], A2, o1b, OP.mult)
        nc.gpsimd.tensor_tensor(t4[:], B2, o2b, OP.mult)
        nc.gpsimd.tensor_tensor(t3[:], t3[:], t4[:], OP.add)
        nc.gpsimd.tensor_tensor(O3[j3][:], t3[:],
                                o7q[:, :, :, j3], OP.add)
    # sequential walk over the 4 sub-blocks: z at superchunks 4*(s2+1)
    for s2 in range(C3 - 1):
        rA = Z3[L3 - 1][:, s2::2 * C3]
        rB = Z3[L3 - 1][:, C3 + s2::2 * C3]
        zdzd = zt[:, 32 * s2:32 * s2 + 1].broadcast_to((D, 2))
        zszs = zt[:, C + 32 * s2:C + 32 * s2 + 1].broadcast_to((D, 2))
        oc = O3[L3 - 1][:, s2::C3]
        u1 = _mk_tile(pt, [D, 2], F32, "l3t1")
        u2 = _mk_tile(pt, [D, 2], F32, "l3t2")
        nc.vector.tensor_tensor(u1[:], rA, zdzd, OP.mult)
        nc.vector.tensor_tensor(u2[:], rB, zszs, OP.mult)
        nc.vector.tensor_tensor(u1[:], u1[:], u2[:], OP.add)
        nc.vector.tensor_tensor(zt[:, 32 * (s2 + 1)::C][:, 0:2], u1[:], oc, OP.add)
    # in-block recon: z at superchunk 4*s2 + j3 + 1 for j3 = 0..2
    for j3 in range(L3 - 1):
        zj4 = Z3[j3][:].rearrange("p (q h) -> p q h", q=4)
        A3 = zj4[:, 0::2, :]
        B3 = zj4[:, 1::2, :]
        zdzd = zt[:, 0:C:32][:, 0:C3].unsqueeze(1).broadcast_to((D, 2, C3))
        zszs = zt[:, C::32][:, 0:C3].unsqueeze(1).broadcast_to((D, 2, C3))
        eng = nc.vector if j3 % 2 == 0 else nc.gpsimd
        v1 = _mk_tile(pt, [D, 2 * C3], F32, "rc3t1")
        v2 = _mk_tile(pt, [D, 2 * C3], F32, "rc3t2")
        eng.tensor_tensor(v1[:], A3, zdzd, OP.mult)
        eng.tensor_tensor(v2[:], B3, zszs, OP.mult)
        eng.tensor_tensor(v1[:], v1[:], v2[:], OP.add)
        eng.tensor_tensor(
            zt[:].rearrange("p (b s2 r) -> p b s2 r", b=2, r=32)[:, :, :, L * (j3 + 1)],
            v1[:].rearrange("p (a b) -> p a b", a=2),
            O3[j3][:].rearrange("p (a b) -> p a b", a=2), OP.add)
    zt2 = zt[:].rearrange("p (b c) -> p b c", b=2)
    for j in range(L2 - 1):
        zj4 = Z[j][:].rearrange("p (q h) -> p q h", q=4)
        A3 = zj4[:, 0::2, :]
        B3 = zj4[:, 1::2, :]
        zdzd = zt[:, 0:C:L][:, 0:C2].unsqueeze(1).broadcast_to((D, 2, C2))
        zszs = zt[:, C::L][:, 0:C2].unsqueeze(1).broadcast_to((D, 2, C2))
        eng = nc.vector if j % 2 == 0 else nc.gpsimd
        v1 = _mk_tile(pt, [D, 2 * C2], F32, "rct1")
        v2 = _mk_tile(pt, [D, 2 * C2], F32, "rct2")
        eng.tensor_tensor(v1[:], A3, zdzd, OP.mult)
        eng.tensor_tensor(v2[:], B3, zszs, OP.mult)
        eng.tensor_tensor(v1[:], v1[:], v2[:], OP.add)
        eng.tensor_tensor(zt2[:, :, j + 1::L], v1[:].rearrange("p (a b) -> p a b", a=2),
                          O_[j][:].rearrange("p (a b) -> p a b", a=2), OP.add)

    # ----- pre-scale final-matmul weights (Pool; fills level-2/3 idle) -----
    kwts = []
    kmts = []
    for i in range(L):
        kwt = _mk_tile(p, [C, D], F32, f"kwt{i}")
        nc.gpsimd.tensor_tensor(kwt[:], wS[:, i:i + 1].broadcast_to((C, D)),
                                k_sl(i), OP.mult)
        kwts.append(kwt)
        kmt = _mk_tile(p, [C, D], F32, f"kmt{i}")
        nc.gpsimd.tensor_tensor(kmt[:], wM[:, i:i + 1].broadcast_to((C, D)),
                                k_sl(i), OP.mult)
        kmts.append(kmt)

    # ------- transpose entry states back to [c, d]: zc = [zd|zs] -------
    zc = _mk_tile(p, [C, 2 * D], F32, "zc")
    for b_ in range(2):
        tp = _mk_tile(ps, [C, D], F32, "mm")
        nc.tensor.transpose(tp[:], zt[:, b_ * C:(b_ + 1) * C], ident)
        nc.scalar.copy(zc[:, b_ * D:(b_ + 1) * D], tp[:])

    # ------- phase C (d slabs) + per-slab y/err + pipelined finals -------
    sfp = _mk_tile(psacc, [D, D], F32, "sfp")
    mfp = _mk_tile(psacc, [D, D], F32, "mfp")
    y_all = _mk_tile(p, [C, S], F32, "y_all")
    err = _mk_tile(p, [C, S], F32, "err")
    y3 = y_d.rearrange("(c i) d -> c i d", i=L)
    d_slabs = []
    for i in range(L):
        if i == 0:
            d_sl = zc[:, 0:D]
        else:
            eng = nc.vector if i % 2 == 0 else nc.gpsimd
            tC = _mk_tile(pt, [C, 2 * D], F32, "pct")
            eng.tensor_tensor(tC[:], PR2[i][:], zc[:], OP.mult)
            tC2 = _mk_tile(pt, [C, D], F32, "pct2")
            eng.tensor_tensor(tC2[:], tC[:, 0:D], tC[:, D:2 * D], OP.add)
            d_sl = _mk_tile(p, [C, D], F32, f"dsl{i}")
            eng.tensor_tensor(d_sl[:], tC2[:], rho[i][:], OP.add)
            d_sl = d_sl[:]
        d_slabs.append(d_sl)
        # y slab -> DMA out (paired)
        ysl = y_all[:, i * D:(i + 1) * D]
        nc.gpsimd.tensor_tensor(ysl, q_sl(i), d_sl, OP.mult)
        if i % 2 == 1:
            nc.sync.dma_start(y3[:, i - 1:i + 1, :],
                              y_all[:, (i - 1) * D:(i + 1) * D]
                              .rearrange("c (i d) -> c i d", i=2))
        # err slab
        esl = err[:, i * D:(i + 1) * D]
        et = _mk_tile(pt, [C, D], F32, "pet")
        nc.vector.tensor_tensor(et[:], k_sl(i), d_sl, OP.mult)
        nc.vector.tensor_tensor(esl, et[:], v_sl(i), OP.subtract)
        nc.tensor.matmul(sfp[:], kwts[i][:], esl, start=(i == 0),
                         stop=(i == L - 1), skip_group_check=True)
        nc.tensor.matmul(mfp[:], kmts[i][:], esl, start=(i == 0),
                         stop=(i == L - 1), skip_group_check=True)
    sfs = _mk_tile(p, [D, D], F32, "sfs")
    mfs = _mk_tile(p, [D, D], F32, "mfs")
    nc.scalar.copy(sfs[:], sfp[:])
    nc.scalar.copy(mfs[:], mfp[:])
    nc.sync.dma_start(sf_d[:], sfs[:])
    nc.sync.dma_start(mf_d[:], mfs[:])


_CACHE = {}


def _get_nc(reps=1):
    if ("nc", reps) in _CACHE:
        return _CACHE[("nc", reps)]
    nc = bacc.Bacc("TRN2", target_bir_lowering=False, debug=False)
    NCONST = 4 * D + 3 + 3 * L
    xT_d = nc.dram_tensor("xT", [D, S], F32, kind="ExternalInput").ap()
    consts_d = nc.dram_tensor("consts", [D, NCONST], F32, kind="ExternalInput").ap()
    y_d = nc.dram_tensor("y", [S, D], F32, kind="ExternalOutput").ap()
    mf_d = nc.dram_tensor("MF", [D, D], F32, kind="ExternalOutput").ap()
    sf_d = nc.dram_tensor("SF", [D, D], F32, kind="ExternalOutput").ap()
    with tile.TileContext(nc) as tc:
        for r in range(reps):
            with ExitStack() as ctx:
                emit(ctx, tc, (y_d, mf_d, sf_d), (xT_d, consts_d),
                     sfx=(f"_r{r}" if r else ""))
    nc.compile()
    _CACHE[("nc", reps)] = nc
    return nc


def make_consts(Wk, Wv, Wq, Wg, bg):
    NCONST = 4 * D + 3 + 3 * L
    cst = np.zeros((D, NCONST), dtype=np.float32)
    cst[:, 0:D] = Wk
    cst[:, D:2 * D] = Wv
    cst[:, 2 * D:3 * D] = Wq
    cst[:, 3 * D:4 * D] = np.eye(D, dtype=np.float32)
    cst[:, 4 * D:4 * D + 3] = Wg
    cst[:, 4 * D + 3:] = np.tile(bg, (D, L))
    return cst


def kernel(x, Wk, Wv, Wq, Wg, bg, _trace=False):
    x = np.asarray(x, dtype=np.float32)
    nc = _get_nc()
    cst = make_consts(np.asarray(Wk, np.float32), np.asarray(Wv, np.float32),
                      np.asarray(Wq, np.float32), np.asarray(Wg, np.float32),
                      np.asarray(bg, np.float32))
    in_maps = []
    for b in range(B):
        in_maps.append({"xT": np.ascontiguousarray(x[b].T), "consts": cst})
    res = run_bass_kernel_spmd(nc, in_maps, core_ids=list(range(B)), trace=_trace)
    y = np.stack([res.results[b]["y"] for b in range(B)])
    MF = np.stack([res.results[b]["MF"] for b in range(B)])
    SF = np.stack([res.results[b]["SF"] for b in range(B)])
    if _trace:
        kernel._last = res
    return (y, (MF, SF))
